# revision 2
# baseline (speedup 1.0000x reference)
"""Locally-connected 2d (3x3, pad 1) + bias + LeakyReLU(0.1) on 8 trn2 cores.

Strategy
--------
out[n, o, oh, ow] = sum_{c,kh,kw} x[n, c, oh+kh-1, ow+kw-1] * W[o, c, oh, ow, kh*3+kw]

The weight (1, 256, 1024, 7, 7, 9) = 462 MB fp32 dominates all traffic and each
element is used exactly N=32 times, so the kernel is HBM-bound.  We:

  * shard out-channels 8-ways (32 per core) so each core streams 1/8 of W,
  * cast W and x to bf16 on the host (halves the dominant traffic),
  * skip (location, tap) pairs that read zero padding (361/441 valid -> -18%),
  * keep x stationary in the PE array (lhsT = x[c_chunk, pixel] of shape
    (K=128 c, M=32 n)) and stream host-packed weight columns through the
    moving port: one matmul per (pixel, c_chunk, kh-tap) covering the
    (ow-window x 32 out-ch) output columns it feeds,
  * accumulate everything in one resident PSUM tile (32 n, 7 oh x 256 cols;
    each oh block is 224 real cols padded to 256 so no matmul crosses a
    2KB PSUM bank),
  * epilogue: DVE add of a host-broadcast bias then LeakyReLU as
    max(0.1*t, t) in a single scalar_tensor_tensor op.

Everything is SPMD-uniform: all per-core differences live in input *content*
(the packed weight / bias), never in shapes or program structure.
"""

import sys

import numpy as np

if "/opt/trn_rl_repo" not in sys.path:
    sys.path.insert(0, "/opt/trn_rl_repo")

import ml_dtypes

# ---------------------------------------------------------------- constants
N = 32
C_IN = 1024
H = W = 7
C_OUT = 256
OH = OW = 7
KH = KW = 3
NCORES = 8
O_SH = C_OUT // NCORES          # 32 out-channels per core
P = 128                          # SBUF partitions
NCHUNK = C_IN // P               # 8 contraction chunks
OH_BLOCK = 256                   # psum cols per oh row (224 real + 32 pad)
REAL_BLOCK = OW * O_SH           # 224
PSUM_COLS = OH * OH_BLOCK        # 1792
OUT_COLS = OH * REAL_BLOCK       # 1568
X_COLS = NCHUNK * H * W * N      # 12544
NEG_SLOPE = 0.1


def _schedule():
    """Per input pixel: valid kh taps and the ascending ow window it feeds."""
    pixels = []
    for ih in range(H):
        for iw in range(W):
            i_list = [i for i in range(KH) if 0 <= ih + 1 - i <= OH - 1]
            ow_list = [ow for ow in range(iw - 1, iw + 2) if 0 <= ow <= OW - 1]
            pixels.append((ih, iw, i_list, ow_list))
    return pixels


_PIXELS = _schedule()
TOTAL_COLS = sum(NCHUNK * len(i) * len(o) * O_SH for _, _, i, o in _PIXELS)  # 92416


# ---------------------------------------------------------------- host packing
def _pack_weight(weight):
    """-> list of 8 arrays (128, TOTAL_COLS) bf16, one per core.

    Column order: pixel-major, then (chunk, kh-tap, ow asc, o).  Row p of
    chunk k holds input channel c = k*128 + p.
    """
    W0 = np.asarray(weight)[0]                                   # (256,1024,7,7,9)
    Wt = np.ascontiguousarray(np.transpose(W0, (1, 0, 2, 3, 4)))  # (c,o,oh,ow,k)
    per_core = [[] for _ in range(NCORES)]
    for ih, iw, i_list, ow_list in _PIXELS:
        ohs, ows, ks = [], [], []
        for i in i_list:
            for ow in ow_list:
                ohs.append(ih + 1 - i)
                ows.append(ow)
                ks.append(i * KW + (iw + 1 - ow))
        B = Wt[:, :, ohs, ows, ks]                    # (1024, 256, npair)
        npair = len(ohs)
        B = B.reshape(NCHUNK, P, C_OUT, npair)
        B = np.transpose(B, (1, 0, 3, 2))             # (p, chunk, pair, o)
        for c in range(NCORES):
            per_core[c].append(
                B[..., c * O_SH:(c + 1) * O_SH].reshape(P, -1))
    return [
        np.ascontiguousarray(np.concatenate(a, axis=1)).astype(ml_dtypes.bfloat16)
        for a in per_core
    ]


def _pack_x(x):
    """-> (128, X_COLS) bf16; free index = (chunk*49 + pixel)*32 + n."""
    xt = np.transpose(np.asarray(x), (1, 2, 3, 0))    # (c, h, w, n)
    xt = xt.reshape(NCHUNK, P, H * W, N)
    xt = np.transpose(xt, (1, 0, 2, 3)).reshape(P, X_COLS)
    return np.ascontiguousarray(xt).astype(ml_dtypes.bfloat16)


def _pack_bias(bias, core):
    b = np.asarray(bias)[0, core * O_SH:(core + 1) * O_SH]   # (32, 7, 7)
    cols = np.transpose(b, (1, 2, 0)).reshape(OUT_COLS)      # (oh, ow, o)
    return np.ascontiguousarray(
        np.broadcast_to(cols[None, :], (N, OUT_COLS))).astype(np.float32)


# ---------------------------------------------------------------- bass program
_PROGRAM = None


def _build_program():
    import concourse.bacc as bacc
    import concourse.tile as tile
    from concourse import mybir

    nc = bacc.Bacc("TRN2", target_bir_lowering=False, debug=False,
                   num_devices=NCORES)
    w_d = nc.dram_tensor("w", [P, TOTAL_COLS], mybir.dt.bfloat16,
                         kind="ExternalInput")
    x_d = nc.dram_tensor("xp", [P, X_COLS], mybir.dt.bfloat16,
                         kind="ExternalInput")
    b_d = nc.dram_tensor("bias", [N, OUT_COLS], mybir.dt.float32,
                         kind="ExternalInput")
    o_d = nc.dram_tensor("out", [N, OUT_COLS], mybir.dt.float32,
                         kind="ExternalOutput")

    with tile.TileContext(nc) as tc:
        with (
            tc.tile_pool(name="cpool", bufs=1) as cpool,
            tc.tile_pool(name="wpool", bufs=6) as wpool,
            tc.tile_pool(name="ppool", bufs=1, space="PSUM") as ppool,
            tc.tile_pool(name="opool", bufs=1) as opool,
        ):
            x_sb = cpool.tile([P, X_COLS], mybir.dt.bfloat16)
            nc.sync.dma_start(x_sb[:], x_d[:])
            bias_sb = cpool.tile([N, OUT_COLS], mybir.dt.float32)
            nc.sync.dma_start(bias_sb[:], b_d[:])
            zero_sb = cpool.tile([P, 512], mybir.dt.bfloat16)
            nc.vector.memset(zero_sb[:], 0.0)

            psum = ppool.tile([N, PSUM_COLS], mybir.dt.float32)

            # Zero-fill each PSUM bank with a full-bank matmul (start=True
            # marks the whole 2KB zero-region; writing all of it leaves no
            # pending-zero bytes, so every later matmul purely accumulates).
            nbank = (PSUM_COLS + 511) // 512
            for b in range(nbank):
                nn = min(512, PSUM_COLS - b * 512)
                nc.tensor.matmul(
                    psum[:, b * 512:b * 512 + nn],
                    zero_sb[:, :N], zero_sb[:, :nn],
                    start=True, stop=False, skip_group_check=True)

            col = 0
            for pix, (ih, iw, i_list, ow_list) in enumerate(_PIXELS):
                wcols = NCHUNK * len(i_list) * len(ow_list) * O_SH
                wt = wpool.tile([P, wcols], mybir.dt.bfloat16, tag="w")
                nc.sync.dma_start(wt[:], w_d[:, col:col + wcols])
                wc = 0
                ncols = len(ow_list) * O_SH
                ow0 = ow_list[0]
                for chunk in range(NCHUNK):
                    s = (chunk * H * W + pix) * N
                    lhs = x_sb[:, s:s + N]
                    for i in i_list:
                        oh = ih + 1 - i
                        off = oh * OH_BLOCK + ow0 * O_SH
                        nc.tensor.matmul(
                            psum[:, off:off + ncols],
                            lhs, wt[:, wc:wc + ncols],
                            start=False, stop=False, skip_group_check=True)
                        wc += ncols
                col += wcols
            assert col == TOTAL_COLS

            # epilogue: t = psum + bias ; out = max(0.1*t, t)
            tmp = opool.tile([N, OUT_COLS], mybir.dt.float32)
            out_sb = opool.tile([N, OUT_COLS], mybir.dt.float32)
            pv = psum[:].rearrange("p (a b) -> p a b", b=OH_BLOCK)[:, :, :REAL_BLOCK]
            tv = tmp[:].rearrange("p (a b) -> p a b", b=REAL_BLOCK)
            bv = bias_sb[:].rearrange("p (a b) -> p a b", b=REAL_BLOCK)
            ov = out_sb[:].rearrange("p (a b) -> p a b", b=REAL_BLOCK)
            nc.vector.tensor_add(tv, pv, bv)
            nc.vector.scalar_tensor_tensor(
                ov, tv, NEG_SLOPE, tv,
                op0=mybir.AluOpType.mult, op1=mybir.AluOpType.max)
            nc.sync.dma_start(o_d[:], out_sb[:])

    nc.finalize()
    return nc


def _get_program():
    global _PROGRAM
    if _PROGRAM is None:
        _PROGRAM = _build_program()
    return _PROGRAM


# ---------------------------------------------------------------- pjrt runner
class _Runner:
    """Compiled SPMD executor with a persistent jit cache.

    Mirrors concourse.bass2jax.run_bass_via_pjrt's multi-core path, but keeps
    the jitted callable (and optionally device-resident inputs) across calls
    so the kernel can be re-executed without re-tracing / re-transferring.
    """

    def __init__(self, nc):
        import jax
        from jax.sharding import Mesh, PartitionSpec
        from jax.experimental.shard_map import shard_map
        from concourse import bass2jax, mybir

        bass2jax.install_neuronx_cc_hook()
        self.jax = jax
        partition_name = (nc.partition_id_tensor.name
                          if nc.partition_id_tensor else None)
        in_names, out_names, out_avals = [], [], []
        zero_outs = []
        for alloc in nc.m.functions[0].allocations:
            if not isinstance(alloc, mybir.MemoryLocationSet):
                continue
            name = alloc.memorylocations[0].name
            if alloc.kind == "ExternalInput":
                if name != partition_name:
                    in_names.append(name)
            elif alloc.kind == "ExternalOutput":
                out_names.append(name)
                shape = tuple(alloc.tensor_shape)
                dtype = mybir.dt.np(alloc.dtype)
                out_avals.append(jax.core.ShapedArray(shape, dtype))
                zero_outs.append(np.zeros(shape, dtype))
        self.in_names = list(in_names)
        self.out_names = out_names
        self.out_avals = out_avals
        self.zero_outs = zero_outs
        n_params = len(in_names)
        n_outs = len(out_avals)
        all_in_names = list(in_names) + list(out_names)
        if partition_name is not None:
            all_in_names.append(partition_name)

        def _body(*args):
            operands = list(args)
            if partition_name is not None:
                operands.append(bass2jax.partition_id_tensor())
            outs = bass2jax._bass_exec_p.bind(
                *operands,
                out_avals=tuple(out_avals),
                in_names=tuple(all_in_names),
                out_names=tuple(out_names),
                lowering_input_output_aliases=(),
                sim_require_finite=True,
                sim_require_nnan=True,
                nc=nc,
            )
            return tuple(outs)

        devices = jax.devices()[:NCORES]
        self.mesh = Mesh(np.asarray(devices), ("core",))
        self.pspec = PartitionSpec("core")
        in_specs = (self.pspec,) * (n_params + n_outs)
        out_specs = (self.pspec,) * n_outs
        donate = tuple(range(n_params, n_params + n_outs))
        self.fn = jax.jit(
            shard_map(_body, mesh=self.mesh, in_specs=in_specs,
                      out_specs=out_specs, check_rep=False),
            donate_argnums=donate, keep_unused=True)

    def stage_inputs(self, in_maps):
        """Concatenate per-core inputs and push them to the devices once."""
        from jax.sharding import NamedSharding
        concat = [
            np.concatenate([np.asarray(in_maps[c][n]) for c in range(NCORES)],
                           axis=0)
            for n in self.in_names
        ]
        sh = NamedSharding(self.mesh, self.pspec)
        return [self.jax.device_put(a, sh) for a in concat]

    def _zeros(self):
        return [np.zeros((NCORES * z.shape[0], *z.shape[1:]), z.dtype)
                for z in self.zero_outs]

    def execute(self, staged):
        outs = self.fn(*staged, *self._zeros())
        return outs

    def results(self, outs):
        out_np = [np.asarray(o) for o in outs]
        return [
            {n: out_np[i].reshape(NCORES, *self.out_avals[i].shape)[c]
             for i, n in enumerate(self.out_names)}
            for c in range(NCORES)
        ]


_RUNNER = None


def _get_runner():
    global _RUNNER
    if _RUNNER is None:
        _RUNNER = _Runner(_get_program())
    return _RUNNER


# ---------------------------------------------------------------- entry points
def _in_maps(inputs):
    w_cores = _pack_weight(inputs["weight"])
    xp = _pack_x(inputs["x"])
    return [
        {"w": w_cores[c], "xp": xp, "bias": _pack_bias(inputs["bias"], c)}
        for c in range(NCORES)
    ]


def _assemble(results):
    parts = []
    for c in range(NCORES):
        o = results[c]["out"].reshape(N, OH, OW, O_SH)
        parts.append(np.transpose(o, (0, 3, 1, 2)))
    return np.concatenate(parts, axis=1).astype(np.float32)


def _run(inputs, trace=False, trace_cores=None):
    r = _get_runner()
    staged = r.stage_inputs(_in_maps(inputs))
    outs = r.execute(staged)
    return _assemble(r.results(outs)), None


def kernel(x, weight, bias):
    out, _ = _run({"x": x, "weight": weight, "bias": bias})
    return out


# revision 5
# speedup vs baseline: 9.8587x; 9.8587x over previous
"""Locally-connected 2d (3x3, pad 1) + bias + LeakyReLU(0.1) on 8 trn2 cores.

Strategy
--------
out[n, o, oh, ow] = sum_{c,kh,kw} x[n, c, oh+kh-1, ow+kw-1] * W[o, c, oh, ow, kh*3+kw]

The weight (1, 256, 1024, 7, 7, 9) = 462 MB fp32 dominates all traffic and each
element is used exactly N=32 times, so the kernel is HBM-bound.  We:

  * shard out-channels 8-ways (32 per core) so each core streams 1/8 of W,
  * cast W and x to bf16 on the host (halves the dominant traffic),
  * skip (location, tap) pairs that read zero padding (361/441 valid -> -18%),
  * keep x stationary in the PE array (lhsT = x[c_chunk, pixel] of shape
    (K=128 c, M=32 n)) and stream host-packed weight columns through the
    moving port: one matmul per (pixel, c_chunk, kh-tap) covering the
    (ow-window x 32 out-ch) output columns it feeds,
  * accumulate everything in one resident PSUM tile (32 n, 7 oh x 256 cols;
    each oh block is 224 real cols padded to 256 so no matmul crosses a
    2KB PSUM bank),
  * epilogue: DVE add of a host-broadcast bias then LeakyReLU as
    max(0.1*t, t) in a single scalar_tensor_tensor op.

Everything is SPMD-uniform: all per-core differences live in input *content*
(the packed weight / bias), never in shapes or program structure.
"""

import sys

import numpy as np

if "/opt/trn_rl_repo" not in sys.path:
    sys.path.insert(0, "/opt/trn_rl_repo")

import ml_dtypes

# ---------------------------------------------------------------- constants
N = 32
C_IN = 1024
H = W = 7
C_OUT = 256
OH = OW = 7
KH = KW = 3
NCORES = 8
O_SH = C_OUT // NCORES          # 32 out-channels per core
P = 128                          # SBUF partitions
NCHUNK = C_IN // P               # 8 contraction chunks
OH_BLOCK = 256                   # psum cols per oh row (224 real + 32 pad)
REAL_BLOCK = OW * O_SH           # 224
PSUM_COLS = OH * OH_BLOCK        # 1792
OUT_COLS = OH * REAL_BLOCK       # 1568
X_COLS = NCHUNK * H * W * N      # 12544
NEG_SLOPE = 0.1


def _schedule():
    """Per input pixel: valid kh taps and the ascending ow window it feeds."""
    pixels = []
    for ih in range(H):
        for iw in range(W):
            i_list = [i for i in range(KH) if 0 <= ih + 1 - i <= OH - 1]
            ow_list = [ow for ow in range(iw - 1, iw + 2) if 0 <= ow <= OW - 1]
            pixels.append((ih, iw, i_list, ow_list))
    return pixels


_PIXELS = _schedule()
TOTAL_COLS = sum(NCHUNK * len(i) * len(o) * O_SH for _, _, i, o in _PIXELS)  # 92416


# ---------------------------------------------------------------- host packing
def _pack_weight(weight):
    """-> list of 8 arrays (128, TOTAL_COLS) bf16, one per core.

    Column order: pixel-major, then (chunk, kh-tap, ow asc, o).  Row p of
    chunk k holds input channel c = k*128 + p.
    """
    W0 = np.asarray(weight)[0]                                   # (256,1024,7,7,9)
    Wt = np.ascontiguousarray(np.transpose(W0, (1, 0, 2, 3, 4)))  # (c,o,oh,ow,k)
    per_core = [[] for _ in range(NCORES)]
    for ih, iw, i_list, ow_list in _PIXELS:
        ohs, ows, ks = [], [], []
        for i in i_list:
            for ow in ow_list:
                ohs.append(ih + 1 - i)
                ows.append(ow)
                ks.append(i * KW + (iw + 1 - ow))
        B = Wt[:, :, ohs, ows, ks]                    # (1024, 256, npair)
        npair = len(ohs)
        B = B.reshape(NCHUNK, P, C_OUT, npair)
        B = np.transpose(B, (1, 0, 3, 2))             # (p, chunk, pair, o)
        for c in range(NCORES):
            per_core[c].append(
                B[..., c * O_SH:(c + 1) * O_SH].reshape(P, -1))
    return [
        np.ascontiguousarray(np.concatenate(a, axis=1)).astype(ml_dtypes.bfloat16)
        for a in per_core
    ]


def _pack_x(x):
    """-> (128, X_COLS) bf16; free index = (chunk*49 + pixel)*32 + n."""
    xt = np.transpose(np.asarray(x), (1, 2, 3, 0))    # (c, h, w, n)
    xt = xt.reshape(NCHUNK, P, H * W, N)
    xt = np.transpose(xt, (1, 0, 2, 3)).reshape(P, X_COLS)
    return np.ascontiguousarray(xt).astype(ml_dtypes.bfloat16)


def _pack_bias(bias, core):
    b = np.asarray(bias)[0, core * O_SH:(core + 1) * O_SH]   # (32, 7, 7)
    cols = np.transpose(b, (1, 2, 0)).reshape(OUT_COLS)      # (oh, ow, o)
    return np.ascontiguousarray(
        np.broadcast_to(cols[None, :], (N, OUT_COLS))).astype(np.float32)


# ---------------------------------------------------------------- bass program
_PROGRAMS = {}


def _build_program(loop_iters=1):
    """loop_iters>1 wraps the whole body in a device-side For_i so that HW
    exec time can be measured by differencing (axon dispatch is ~100ms)."""
    import contextlib

    import concourse.bacc as bacc
    import concourse.tile as tile
    from concourse import mybir

    nc = bacc.Bacc("TRN2", target_bir_lowering=False, debug=False,
                   num_devices=NCORES)
    w_d = nc.dram_tensor("w", [P, TOTAL_COLS], mybir.dt.bfloat16,
                         kind="ExternalInput")
    x_d = nc.dram_tensor("xp", [P, X_COLS], mybir.dt.bfloat16,
                         kind="ExternalInput")
    b_d = nc.dram_tensor("bias", [N, OUT_COLS], mybir.dt.float32,
                         kind="ExternalInput")
    o_d = nc.dram_tensor("out", [N, OUT_COLS], mybir.dt.float32,
                         kind="ExternalOutput")

    with tile.TileContext(nc) as tc:
        with (
            tc.tile_pool(name="cpool", bufs=1) as cpool,
            tc.tile_pool(name="wpool", bufs=6) as wpool,
            tc.tile_pool(name="ppool", bufs=1, space="PSUM") as ppool,
            tc.tile_pool(name="opool", bufs=1) as opool,
        ):
            x_sb = cpool.tile([P, X_COLS], mybir.dt.bfloat16)
            nc.sync.dma_start(x_sb[:], x_d[:])
            bias_sb = cpool.tile([N, OUT_COLS], mybir.dt.float32)
            nc.sync.dma_start(bias_sb[:], b_d[:])
            zero_sb = cpool.tile([P, 512], mybir.dt.bfloat16)
            nc.vector.memset(zero_sb[:], 0.0)

            if loop_iters > 1:
                loop_cm = tc.For_i(0, loop_iters, 1,
                                   hint_engines=(mybir.EngineType.PE,))
            else:
                loop_cm = contextlib.nullcontext()

            with loop_cm:
                psum = ppool.tile([N, PSUM_COLS], mybir.dt.float32)

                # Zero-fill each PSUM bank with a full-bank matmul (start=True
                # marks the whole 2KB zero-region; writing all of it leaves no
                # pending-zero bytes, so later matmuls purely accumulate).
                nbank = (PSUM_COLS + 511) // 512
                for b in range(nbank):
                    nn = min(512, PSUM_COLS - b * 512)
                    nc.tensor.matmul(
                        psum[:, b * 512:b * 512 + nn],
                        zero_sb[:, :N], zero_sb[:, :nn],
                        start=True, stop=False, skip_group_check=True)

                col = 0
                for pix, (ih, iw, i_list, ow_list) in enumerate(_PIXELS):
                    wcols = NCHUNK * len(i_list) * len(ow_list) * O_SH
                    wt = wpool.tile([P, wcols], mybir.dt.bfloat16, tag="w")
                    nc.sync.dma_start(wt[:], w_d[:, col:col + wcols])
                    wc = 0
                    ncols = len(ow_list) * O_SH
                    ow0 = ow_list[0]
                    for chunk in range(NCHUNK):
                        s = (chunk * H * W + pix) * N
                        lhs = x_sb[:, s:s + N]
                        for i in i_list:
                            oh = ih + 1 - i
                            off = oh * OH_BLOCK + ow0 * O_SH
                            nc.tensor.matmul(
                                psum[:, off:off + ncols],
                                lhs, wt[:, wc:wc + ncols],
                                start=False, stop=False, skip_group_check=True)
                            wc += ncols
                    col += wcols
                assert col == TOTAL_COLS

                # epilogue: t = psum + bias ; out = max(0.1*t, t)
                tmp = opool.tile([N, OUT_COLS], mybir.dt.float32)
                out_sb = opool.tile([N, OUT_COLS], mybir.dt.float32)
                pv = psum[:].rearrange("p (a b) -> p a b",
                                       b=OH_BLOCK)[:, :, :REAL_BLOCK]
                tv = tmp[:].rearrange("p (a b) -> p a b", b=REAL_BLOCK)
                bv = bias_sb[:].rearrange("p (a b) -> p a b", b=REAL_BLOCK)
                ov = out_sb[:].rearrange("p (a b) -> p a b", b=REAL_BLOCK)
                nc.vector.tensor_add(tv, pv, bv)
                nc.vector.scalar_tensor_tensor(
                    ov, tv, NEG_SLOPE, tv,
                    op0=mybir.AluOpType.mult, op1=mybir.AluOpType.max)
                nc.sync.dma_start(o_d[:], out_sb[:])

    nc.finalize()
    return nc


def _get_program(loop_iters=1):
    if loop_iters not in _PROGRAMS:
        _PROGRAMS[loop_iters] = _build_program(loop_iters)
    return _PROGRAMS[loop_iters]


# ---------------------------------------------------------------- pjrt runner
class _Runner:
    """Compiled SPMD executor with a persistent jit cache.

    Mirrors concourse.bass2jax.run_bass_via_pjrt's multi-core path, but keeps
    the jitted callable (and optionally device-resident inputs) across calls
    so the kernel can be re-executed without re-tracing / re-transferring.
    """

    def __init__(self, nc):
        import jax
        from jax.sharding import Mesh, PartitionSpec
        from jax.experimental.shard_map import shard_map
        from concourse import bass2jax, mybir

        bass2jax.install_neuronx_cc_hook()
        self.jax = jax
        partition_name = (nc.partition_id_tensor.name
                          if nc.partition_id_tensor else None)
        in_names, out_names, out_avals = [], [], []
        zero_outs = []
        for alloc in nc.m.functions[0].allocations:
            if not isinstance(alloc, mybir.MemoryLocationSet):
                continue
            name = alloc.memorylocations[0].name
            if alloc.kind == "ExternalInput":
                if name != partition_name:
                    in_names.append(name)
            elif alloc.kind == "ExternalOutput":
                out_names.append(name)
                shape = tuple(alloc.tensor_shape)
                dtype = mybir.dt.np(alloc.dtype)
                out_avals.append(jax.core.ShapedArray(shape, dtype))
                zero_outs.append(np.zeros(shape, dtype))
        self.in_names = list(in_names)
        self.out_names = out_names
        self.out_avals = out_avals
        self.zero_outs = zero_outs
        n_params = len(in_names)
        n_outs = len(out_avals)
        all_in_names = list(in_names) + list(out_names)
        if partition_name is not None:
            all_in_names.append(partition_name)

        def _body(*args):
            operands = list(args)
            if partition_name is not None:
                operands.append(bass2jax.partition_id_tensor())
            outs = bass2jax._bass_exec_p.bind(
                *operands,
                out_avals=tuple(out_avals),
                in_names=tuple(all_in_names),
                out_names=tuple(out_names),
                lowering_input_output_aliases=(),
                sim_require_finite=True,
                sim_require_nnan=True,
                nc=nc,
            )
            return tuple(outs)

        devices = jax.devices()[:NCORES]
        self.mesh = Mesh(np.asarray(devices), ("core",))
        self.pspec = PartitionSpec("core")
        in_specs = (self.pspec,) * (n_params + n_outs)
        out_specs = (self.pspec,) * n_outs
        # No donation: the kernel writes every element of its outputs, so the
        # (required-by-signature) zero buffers are never actually read and can
        # stay device-resident across calls.
        self.fn = jax.jit(
            shard_map(_body, mesh=self.mesh, in_specs=in_specs,
                      out_specs=out_specs, check_rep=False),
            keep_unused=True)

    def stage_inputs(self, in_maps):
        """Concatenate per-core inputs and push them to the devices once."""
        from jax.sharding import NamedSharding
        concat = [
            np.concatenate([np.asarray(in_maps[c][n]) for c in range(NCORES)],
                           axis=0)
            for n in self.in_names
        ]
        concat += [np.zeros((NCORES * z.shape[0], *z.shape[1:]), z.dtype)
                   for z in self.zero_outs]
        sh = NamedSharding(self.mesh, self.pspec)
        return [self.jax.device_put(a, sh) for a in concat]

    def execute(self, staged):
        outs = self.fn(*staged)
        return outs

    def results(self, outs):
        out_np = [np.asarray(o) for o in outs]
        return [
            {n: out_np[i].reshape(NCORES, *self.out_avals[i].shape)[c]
             for i, n in enumerate(self.out_names)}
            for c in range(NCORES)
        ]


_RUNNERS = {}


def _get_runner(loop_iters=1):
    if loop_iters not in _RUNNERS:
        _RUNNERS[loop_iters] = _Runner(_get_program(loop_iters))
    return _RUNNERS[loop_iters]


# ---------------------------------------------------------------- entry points
def _in_maps(inputs):
    w_cores = _pack_weight(inputs["weight"])
    xp = _pack_x(inputs["x"])
    return [
        {"w": w_cores[c], "xp": xp, "bias": _pack_bias(inputs["bias"], c)}
        for c in range(NCORES)
    ]


def _assemble(results):
    parts = []
    for c in range(NCORES):
        o = results[c]["out"].reshape(N, OH, OW, O_SH)
        parts.append(np.transpose(o, (0, 3, 1, 2)))
    return np.concatenate(parts, axis=1).astype(np.float32)


def _run(inputs, trace=False, trace_cores=None):
    r = _get_runner()
    staged = r.stage_inputs(_in_maps(inputs))
    outs = r.execute(staged)
    return _assemble(r.results(outs)), None


def kernel(x, weight, bias):
    out, _ = _run({"x": x, "weight": weight, "bias": bias})
    return out


# revision 7
# speedup vs baseline: 10.9537x; 1.1111x over previous
"""Locally-connected 2d (3x3, pad 1) + bias + LeakyReLU(0.1) on 8 trn2 cores.

Strategy
--------
out[n, o, oh, ow] = sum_{c,kh,kw} x[n, c, oh+kh-1, ow+kw-1] * W[o, c, oh, ow, kh*3+kw]

The weight (1, 256, 1024, 7, 7, 9) = 462 MB fp32 dominates all traffic and each
element is used exactly N=32 times, so the kernel is HBM-bound.  We:

  * shard out-channels 8-ways (32 per core) so each core streams 1/8 of W,
  * cast W and x to bf16 on the host (halves the dominant traffic),
  * skip (location, tap) pairs that read zero padding (361/441 valid -> -18%),
  * keep x stationary in the PE array (lhsT = x[c_chunk, pixel] of shape
    (K=128 c, M=32 n)) and stream host-packed weight columns through the
    moving port: one matmul per (pixel, c_chunk, kh-tap) covering the
    (ow-window x 32 out-ch) output columns it feeds,
  * accumulate everything in one resident PSUM tile (32 n, 7 oh x 256 cols;
    each oh block is 224 real cols padded to 256 so no matmul crosses a
    2KB PSUM bank),
  * epilogue: DVE add of a host-broadcast bias then LeakyReLU as
    max(0.1*t, t) in a single scalar_tensor_tensor op.

Everything is SPMD-uniform: all per-core differences live in input *content*
(the packed weight / bias), never in shapes or program structure.
"""

import sys

import numpy as np

if "/opt/trn_rl_repo" not in sys.path:
    sys.path.insert(0, "/opt/trn_rl_repo")

import ml_dtypes

# ---------------------------------------------------------------- constants
N = 32
C_IN = 1024
H = W = 7
C_OUT = 256
OH = OW = 7
KH = KW = 3
NCORES = 8
O_SH = C_OUT // NCORES          # 32 out-channels per core
P = 128                          # SBUF partitions
NCHUNK = C_IN // P               # 8 contraction chunks
OH_BLOCK = 256                   # psum cols per oh row (224 real + 32 pad)
REAL_BLOCK = OW * O_SH           # 224
PSUM_COLS = OH * OH_BLOCK        # 1792
OUT_COLS = OH * REAL_BLOCK       # 1568
X_COLS = NCHUNK * H * W * N      # 12544
NEG_SLOPE = 0.1


def _schedule():
    """Per input pixel: valid kh taps and the ascending ow window it feeds."""
    pixels = []
    for ih in range(H):
        for iw in range(W):
            i_list = [i for i in range(KH) if 0 <= ih + 1 - i <= OH - 1]
            ow_list = [ow for ow in range(iw - 1, iw + 2) if 0 <= ow <= OW - 1]
            pixels.append((ih, iw, i_list, ow_list))
    return pixels


_PIXELS = _schedule()
TOTAL_COLS = sum(NCHUNK * len(i) * len(o) * O_SH for _, _, i, o in _PIXELS)  # 92416


# ---------------------------------------------------------------- host packing
def _pack_weight(weight):
    """-> list of 8 arrays (128, TOTAL_COLS) bf16, one per core.

    Column order: pixel-major, then (chunk, kh-tap, ow asc, o).  Row p of
    chunk k holds input channel c = k*128 + p.
    """
    W0 = np.asarray(weight)[0]                                   # (256,1024,7,7,9)
    Wt = np.ascontiguousarray(np.transpose(W0, (1, 0, 2, 3, 4)))  # (c,o,oh,ow,k)
    per_core = [[] for _ in range(NCORES)]
    for ih, iw, i_list, ow_list in _PIXELS:
        ohs, ows, ks = [], [], []
        for i in i_list:
            for ow in ow_list:
                ohs.append(ih + 1 - i)
                ows.append(ow)
                ks.append(i * KW + (iw + 1 - ow))
        B = Wt[:, :, ohs, ows, ks]                    # (1024, 256, npair)
        npair = len(ohs)
        B = B.reshape(NCHUNK, P, C_OUT, npair)
        B = np.transpose(B, (1, 0, 3, 2))             # (p, chunk, pair, o)
        for c in range(NCORES):
            per_core[c].append(
                B[..., c * O_SH:(c + 1) * O_SH].reshape(P, -1))
    return [
        np.ascontiguousarray(np.concatenate(a, axis=1)).astype(ml_dtypes.bfloat16)
        for a in per_core
    ]


def _pack_x(x):
    """-> (128, X_COLS) bf16; free index = (chunk*49 + pixel)*32 + n."""
    xt = np.transpose(np.asarray(x), (1, 2, 3, 0))    # (c, h, w, n)
    xt = xt.reshape(NCHUNK, P, H * W, N)
    xt = np.transpose(xt, (1, 0, 2, 3)).reshape(P, X_COLS)
    return np.ascontiguousarray(xt).astype(ml_dtypes.bfloat16)


def _pack_bias(bias, core):
    b = np.asarray(bias)[0, core * O_SH:(core + 1) * O_SH]   # (32, 7, 7)
    cols = np.transpose(b, (1, 2, 0)).reshape(OUT_COLS)      # (oh, ow, o)
    return np.ascontiguousarray(
        np.broadcast_to(cols[None, :], (N, OUT_COLS))).astype(np.float32)


# ---------------------------------------------------------------- bass program
_PROGRAMS = {}


def _build_program(loop_iters=1):
    """loop_iters>1 wraps the whole body in a device-side For_i so that HW
    exec time can be measured by differencing (axon dispatch is ~100ms)."""
    import contextlib

    import concourse.bacc as bacc
    import concourse.tile as tile
    from concourse import mybir

    nc = bacc.Bacc("TRN2", target_bir_lowering=False, debug=False,
                   num_devices=NCORES)
    w_d = nc.dram_tensor("w", [P, TOTAL_COLS], mybir.dt.bfloat16,
                         kind="ExternalInput")
    x_d = nc.dram_tensor("xp", [P, X_COLS], mybir.dt.bfloat16,
                         kind="ExternalInput")
    b_d = nc.dram_tensor("bias", [N, OUT_COLS], mybir.dt.float32,
                         kind="ExternalInput")
    o_d = nc.dram_tensor("out", [N, OUT_COLS], mybir.dt.float32,
                         kind="ExternalOutput")

    with tile.TileContext(nc) as tc:
        with (
            tc.tile_pool(name="cpool", bufs=1) as cpool,
            tc.tile_pool(name="wpool", bufs=6) as wpool,
            tc.tile_pool(name="ppool", bufs=1, space="PSUM") as ppool,
            tc.tile_pool(name="opool", bufs=1) as opool,
        ):
            x_sb = cpool.tile([P, X_COLS], mybir.dt.bfloat16)
            nc.sync.dma_start(x_sb[:], x_d[:])
            bias_sb = cpool.tile([N, OUT_COLS], mybir.dt.float32)
            nc.sync.dma_start(bias_sb[:], b_d[:])
            zero_sb = cpool.tile([P, 512], mybir.dt.bfloat16)
            nc.vector.memset(zero_sb[:], 0.0)

            if loop_iters > 1:
                loop_cm = tc.For_i(0, loop_iters, 1,
                                   hint_engines=(mybir.EngineType.PE,))
            else:
                loop_cm = contextlib.nullcontext()

            with loop_cm:
                # one PSUM tile per output row -> per-row dependency tracking,
                # so each row's epilogue overlaps the remaining weight stream
                psums = [ppool.tile([N, OH_BLOCK], mybir.dt.float32,
                                    name=f"psum{oh}", tag=f"psum{oh}")
                         for oh in range(OH)]

                # Zero-fill each PSUM tile with a matmul (start=True marks the
                # whole 2KB zero-region as pending-zero; writing the tile's
                # 256 cols clears every byte later matmuls touch, so they
                # purely accumulate).
                for oh in range(OH):
                    nc.tensor.matmul(
                        psums[oh][:, :OH_BLOCK],
                        zero_sb[:, :N], zero_sb[:, :OH_BLOCK],
                        start=True, stop=False, skip_group_check=True)

                tmp = opool.tile([N, OUT_COLS], mybir.dt.float32)
                out_sb = opool.tile([N, OUT_COLS], mybir.dt.float32)

                def epilogue(oh):
                    # t = psum + bias ; out = max(0.1*t, t)
                    pv = psums[oh][:, :REAL_BLOCK]
                    tv = tmp[:, oh * REAL_BLOCK:(oh + 1) * REAL_BLOCK]
                    bv = bias_sb[:, oh * REAL_BLOCK:(oh + 1) * REAL_BLOCK]
                    ov = out_sb[:, oh * REAL_BLOCK:(oh + 1) * REAL_BLOCK]
                    nc.vector.tensor_add(tv, pv, bv)
                    nc.vector.scalar_tensor_tensor(
                        ov, tv, NEG_SLOPE, tv,
                        op0=mybir.AluOpType.mult, op1=mybir.AluOpType.max)
                    nc.sync.dma_start(
                        o_d[:, oh * REAL_BLOCK:(oh + 1) * REAL_BLOCK], ov)

                col = 0
                for pix, (ih, iw, i_list, ow_list) in enumerate(_PIXELS):
                    wcols = NCHUNK * len(i_list) * len(ow_list) * O_SH
                    wt = wpool.tile([P, wcols], mybir.dt.bfloat16, tag="w")
                    nc.sync.dma_start(wt[:], w_d[:, col:col + wcols])
                    wc = 0
                    ncols = len(ow_list) * O_SH
                    ow0 = ow_list[0]
                    for chunk in range(NCHUNK):
                        s = (chunk * H * W + pix) * N
                        lhs = x_sb[:, s:s + N]
                        for i in i_list:
                            oh = ih + 1 - i
                            nc.tensor.matmul(
                                psums[oh][:, ow0 * O_SH:ow0 * O_SH + ncols],
                                lhs, wt[:, wc:wc + ncols],
                                start=False, stop=False, skip_group_check=True)
                            wc += ncols
                    col += wcols
                    if iw == W - 1:
                        # row ih done: output row ih-1 is complete
                        if ih >= 1:
                            epilogue(ih - 1)
                        if ih == H - 1:
                            epilogue(ih)
                assert col == TOTAL_COLS

    nc.finalize()
    return nc


def _get_program(loop_iters=1):
    if loop_iters not in _PROGRAMS:
        _PROGRAMS[loop_iters] = _build_program(loop_iters)
    return _PROGRAMS[loop_iters]


# ---------------------------------------------------------------- pjrt runner
class _Runner:
    """Compiled SPMD executor with a persistent jit cache.

    Mirrors concourse.bass2jax.run_bass_via_pjrt's multi-core path, but keeps
    the jitted callable (and optionally device-resident inputs) across calls
    so the kernel can be re-executed without re-tracing / re-transferring.
    """

    def __init__(self, nc):
        import jax
        from jax.sharding import Mesh, PartitionSpec
        from jax.experimental.shard_map import shard_map
        from concourse import bass2jax, mybir

        bass2jax.install_neuronx_cc_hook()
        self.jax = jax
        partition_name = (nc.partition_id_tensor.name
                          if nc.partition_id_tensor else None)
        in_names, out_names, out_avals = [], [], []
        zero_outs = []
        for alloc in nc.m.functions[0].allocations:
            if not isinstance(alloc, mybir.MemoryLocationSet):
                continue
            name = alloc.memorylocations[0].name
            if alloc.kind == "ExternalInput":
                if name != partition_name:
                    in_names.append(name)
            elif alloc.kind == "ExternalOutput":
                out_names.append(name)
                shape = tuple(alloc.tensor_shape)
                dtype = mybir.dt.np(alloc.dtype)
                out_avals.append(jax.core.ShapedArray(shape, dtype))
                zero_outs.append(np.zeros(shape, dtype))
        self.in_names = list(in_names)
        self.out_names = out_names
        self.out_avals = out_avals
        self.zero_outs = zero_outs
        n_params = len(in_names)
        n_outs = len(out_avals)
        all_in_names = list(in_names) + list(out_names)
        if partition_name is not None:
            all_in_names.append(partition_name)

        def _body(*args):
            operands = list(args)
            if partition_name is not None:
                operands.append(bass2jax.partition_id_tensor())
            outs = bass2jax._bass_exec_p.bind(
                *operands,
                out_avals=tuple(out_avals),
                in_names=tuple(all_in_names),
                out_names=tuple(out_names),
                lowering_input_output_aliases=(),
                sim_require_finite=True,
                sim_require_nnan=True,
                nc=nc,
            )
            return tuple(outs)

        devices = jax.devices()[:NCORES]
        self.mesh = Mesh(np.asarray(devices), ("core",))
        self.pspec = PartitionSpec("core")
        in_specs = (self.pspec,) * (n_params + n_outs)
        out_specs = (self.pspec,) * n_outs
        # No donation: the kernel writes every element of its outputs, so the
        # (required-by-signature) zero buffers are never actually read and can
        # stay device-resident across calls.
        self.fn = jax.jit(
            shard_map(_body, mesh=self.mesh, in_specs=in_specs,
                      out_specs=out_specs, check_rep=False),
            keep_unused=True)

    def stage_inputs(self, in_maps):
        """Concatenate per-core inputs and push them to the devices once."""
        from jax.sharding import NamedSharding
        concat = [
            np.concatenate([np.asarray(in_maps[c][n]) for c in range(NCORES)],
                           axis=0)
            for n in self.in_names
        ]
        concat += [np.zeros((NCORES * z.shape[0], *z.shape[1:]), z.dtype)
                   for z in self.zero_outs]
        sh = NamedSharding(self.mesh, self.pspec)
        return [self.jax.device_put(a, sh) for a in concat]

    def execute(self, staged):
        outs = self.fn(*staged)
        return outs

    def results(self, outs):
        out_np = [np.asarray(o) for o in outs]
        return [
            {n: out_np[i].reshape(NCORES, *self.out_avals[i].shape)[c]
             for i, n in enumerate(self.out_names)}
            for c in range(NCORES)
        ]


_RUNNERS = {}


def _get_runner(loop_iters=1):
    if loop_iters not in _RUNNERS:
        _RUNNERS[loop_iters] = _Runner(_get_program(loop_iters))
    return _RUNNERS[loop_iters]


# ---------------------------------------------------------------- entry points
def _in_maps(inputs):
    w_cores = _pack_weight(inputs["weight"])
    xp = _pack_x(inputs["x"])
    return [
        {"w": w_cores[c], "xp": xp, "bias": _pack_bias(inputs["bias"], c)}
        for c in range(NCORES)
    ]


def _assemble(results):
    parts = []
    for c in range(NCORES):
        o = results[c]["out"].reshape(N, OH, OW, O_SH)
        parts.append(np.transpose(o, (0, 3, 1, 2)))
    return np.concatenate(parts, axis=1).astype(np.float32)


def _run(inputs, trace=False, trace_cores=None):
    r = _get_runner()
    staged = r.stage_inputs(_in_maps(inputs))
    outs = r.execute(staged)
    return _assemble(r.results(outs)), None


def kernel(x, weight, bias):
    out, _ = _run({"x": x, "weight": weight, "bias": bias})
    return out
